# revision 1
# baseline (speedup 1.0000x reference)
"""Trainium2 Bass kernel for per-component tiny-MLP stack (CFCN constructor).

Computation (reference):
    h1 = relu(x[:, :, None] * W1 + b1)            # [B, D, H]
    h2 = relu(einsum('bdh,dhk->bdk', h1, W2) + b2)  # [B, D, H]
    out = einsum('bdh,dh->bd', h2, W3) + b3        # [B, D]

B=16384, D=64, H=128. Sharded over 8 NeuronCores by component: 8 components
per core, full batch per core (fully independent per-component MLPs — no
collectives needed).

Per-core dataflow (everything transposed: H on partitions, batch on free):
  L1: PE outer-product matmuls, K=2 (x row + ones row) so the bias rides in
      the contraction; 2-way row-strip packing (tile_position) so two
      components' L1 matmuls run concurrently.
  h1 = relu(z1): PSUM->SBUF eviction on ScalarE/VectorE (alternating).
  L2: K=128 fp32r matmul with W2_d stationary.
  h2 = relu(z2 + b2): eviction with per-partition bias.
  L3: per batch window, 8 accumulating M=8 fp32r matmuls (stationary =
      [128, 8] with only column d nonzero = W3_d) sum into one PSUM bank with
      the 8 components' outputs on contiguous partitions 0..7, evicted as a
      legal [8, 512] engine copy and DMA'd out.
  b3 and final transpose applied on host.
"""

import sys

if "/opt/trn_rl_repo" not in sys.path:
    sys.path.insert(0, "/opt/trn_rl_repo")

import numpy as np

B, D, H = 16384, 64, 128
NCORES = 8
DPC = D // NCORES  # components per core = 8
P = 128
W_ = 512           # batch window per matmul (fp32 moving-operand max)
BT = 2048          # batch chunk per xones tile
NBT = B // BT      # 8
NG = DPC // 2      # component pairs per core = 4

_CACHE = {}


def _build_program():
    from collections import deque
    from contextlib import ExitStack

    from concourse import bass, mybir
    from concourse import tile
    from concourse.tile_rust import add_dep_helper

    f32 = mybir.dt.float32
    f32r = mybir.dt.float32r
    Relu = mybir.ActivationFunctionType.Relu
    Copy = mybir.ActivationFunctionType.Copy
    Alu = mybir.AluOpType

    nc = bass.Bass("TRN2", target_bir_lowering=False, debug=False)

    # DRAM I/O (per-core data supplied via in_maps)
    xa = nc.dram_tensor("xa", [2 * DPC, B], f32r, kind="ExternalInput")
    wpk = nc.dram_tensor("wpk", [P, NG * H], f32r, kind="ExternalInput")
    w2 = nc.dram_tensor("w2", [H, DPC * H], f32r, kind="ExternalInput")
    b2t = nc.dram_tensor("b2t", [H, DPC], f32, kind="ExternalInput")
    # W3 embeddings: w3e[:, 8*d : 8*d+8] is [H, 8] with only column d nonzero
    w3e = nc.dram_tensor("w3e", [H, DPC * DPC], f32r, kind="ExternalInput")
    # [bt, w, d, 512] so the per-(bt,w) staging tile DMAs out with a natural AP
    o = nc.dram_tensor("o", [NBT, BT // W_, DPC, W_], f32, kind="ExternalOutput")

    ecnt = [0]

    with tile.TileContext(nc) as tc, ExitStack() as ctx:
        wts = ctx.enter_context(tc.tile_pool(name="wts", bufs=1))
        xo_pool = ctx.enter_context(tc.tile_pool(name="xo", bufs=3))
        z1_pool = ctx.enter_context(tc.tile_pool(name="z1", bufs=2, space="PSUM"))
        h1_pool = ctx.enter_context(tc.tile_pool(name="h1", bufs=4))
        z2_pool = ctx.enter_context(tc.tile_pool(name="z2", bufs=2, space="PSUM"))
        h2_pool = ctx.enter_context(tc.tile_pool(name="h2", bufs=18))
        ost_pool = ctx.enter_context(tc.tile_pool(name="ost", bufs=3))

        wpk_sb = wts.tile([P, NG * H], f32r)
        wd_wpk = nc.sync.dma_start(wpk_sb[:], wpk[:, :])
        w2_sb = wts.tile([H, DPC * H], f32r)
        wd_w2 = nc.sync.dma_start(w2_sb[:], w2[:, :])
        b2_sb = wts.tile([H, DPC], f32)
        wd_b2 = nc.sync.dma_start(b2_sb[:], b2t[:, :])
        w3_sb = wts.tile([H, DPC * DPC], f32r)
        wd_w3 = nc.sync.dma_start(w3_sb[:], w3e[:, :])

        def evict(dst, src, bias_col, use_act=None):
            # dst = relu(src + bias); alternate ScalarE (5/9) and VectorE (4/9)
            # to balance the two engines' eviction throughput.
            if use_act is None:
                use_act = (ecnt[0] * 5) % 9 < 5
            ecnt[0] += 1
            if use_act:
                if bias_col is None:
                    return nc.scalar.activation(dst[:], src[:], Relu)
                return nc.scalar.activation(dst[:], src[:], Relu, bias=bias_col)
            if bias_col is None:
                return nc.vector.tensor_scalar(dst[:], src[:], 0.0, None, Alu.max)
            return nc.vector.tensor_scalar(
                dst[:], src[:], bias_col, 0.0, Alu.add, Alu.max
            )

        def evict_copy(dst, src, use_act=None):
            if use_act is None:
                use_act = (ecnt[0] * 5) % 9 < 5
            ecnt[0] += 1
            if use_act:
                return nc.scalar.activation(dst[:], src[:], Copy)
            return nc.vector.tensor_copy(dst[:], src[:])

        # Self-loading fp32r matmuls only have ONE sync-wait slot in walrus
        # codegen. Absorb extra cross-engine waits into PE nops placed just
        # before each matmul group: the nop waits, the PE FIFO order covers
        # the matmul, and Tile's vector clock elides the duplicate wait.
        def pe_absorb(producers):
            # returns nops (created BEFORE the matmuls they shield) — caller
            # must order the first matmul after them via pe_order()
            nops = []
            for p in producers:
                if p is None:
                    continue
                n = nc.tensor.nop()
                add_dep_helper(n.ins, p.ins, True, "wait-carrier")
                nops.append(n)
            return nops

        def pe_order(first_mm, nops):
            for n in nops:
                add_dep_helper(first_mm.ins, n.ins, False, "carrier-order")

        # slot-freeing instruction trackers (bufs=2 pools)
        zslot = deque(maxlen=2)   # tag "z1" (z1 tiles + op tiles)
        z2slot = deque(maxlen=2)  # z2 tiles

        wdmas = [wd_wpk, wd_w2, wd_b2, wd_w3]
        for bt in range(NBT):
            h2s = {}
            for g in range(NG):
                xo = xo_pool.tile([P, BT], f32r)
                sl = slice(bt * BT, (bt + 1) * BT)
                xdA = nc.sync.dma_start(xo[0:2, :], xa[4 * g : 4 * g + 2, sl])
                xdB = nc.sync.dma_start(xo[32:34, :], xa[4 * g + 2 : 4 * g + 4, sl])
                xo_deps = [xdA, xdB] + wdmas
                wdmas = []

                for wp in range(2):
                    unit_act = (2 * g + wp) % 2 == 0
                    h1s = []
                    for q in range(2):
                        w = 2 * wp + q
                        deps = list(xo_deps)
                        xo_deps = []
                        if len(zslot) == zslot.maxlen:
                            deps.append(zslot[0])
                        nops = pe_absorb(deps)
                        z1 = z1_pool.tile([P, 2 * W_], f32)
                        mm0 = None
                        for s in range(2):
                            mm = nc.tensor.matmul(
                                z1[:, s * W_ : (s + 1) * W_],
                                lhsT=wpk_sb[32 * s : 32 * s + 2, g * H : (g + 1) * H],
                                rhs=xo[32 * s : 32 * s + 2, w * W_ : (w + 1) * W_],
                                start=True,
                                stop=True,
                                tile_position=(32 * s, 0),
                            )
                            mm0 = mm0 or mm
                        pe_order(mm0, nops)
                        h1 = h1_pool.tile([P, 2 * W_], f32r)
                        zslot.append(evict(h1, z1, None, use_act=unit_act))
                        h1s.append(h1)
                    for s in range(2):
                        di = 2 * g + s
                        nops = (
                            pe_absorb([z2slot[0]])
                            if len(z2slot) == z2slot.maxlen
                            else []
                        )
                        z2 = z2_pool.tile([P, 2 * W_], f32)
                        mm0 = None
                        for q in range(2):
                            mm = nc.tensor.matmul(
                                z2[:, q * W_ : (q + 1) * W_],
                                lhsT=w2_sb[:, di * H : (di + 1) * H],
                                rhs=h1s[q][:, s * W_ : (s + 1) * W_],
                                start=True,
                                stop=True,
                            )
                            mm0 = mm0 or mm
                        pe_order(mm0, nops)
                        h2 = h2_pool.tile([P, 2 * W_], f32r)
                        z2slot.append(
                            evict(h2, z2, b2_sb[:, di : di + 1], use_act=unit_act)
                        )
                        h2s[(di, wp)] = h2

            # L3 for the whole bt chunk: for each 512-window accumulate all 8
            # components into PSUM partitions 0..7 (W3-embedding stationaries).
            for w in range(4):
                wp, q = w // 2, w % 2
                # share the z1 pool's PSUM slots — op is tiny and the L3
                # phase interleaves with the next bt's L1 fills
                nops = pe_absorb([zslot[0]]) if len(zslot) == zslot.maxlen else []
                op = z1_pool.tile([DPC, W_], f32, tag="z1")
                mm0 = None
                for di in range(DPC):
                    mm = nc.tensor.matmul(
                        op[:, :],
                        lhsT=w3_sb[:, DPC * di : DPC * (di + 1)],
                        rhs=h2s[(di, wp)][:, q * W_ : (q + 1) * W_],
                        start=(di == 0),
                        stop=(di == DPC - 1),
                    )
                    mm0 = mm0 or mm
                pe_order(mm0, nops)
                ost = ost_pool.tile([DPC, W_], f32)
                zslot.append(evict_copy(ost, op[:]))
                nc.sync.dma_start(o[bt, w], ost[:])

    return nc


def _build_program_raw():
    """Raw-bass build: manual engine streams + counting semaphores.

    Self-loading fp32r matmuls only support ONE sync-wait in walrus codegen,
    so all multi-proc waits are standalone wait_ge instructions placed by
    hand. PSUM: 4 ping-pong pairs of [128, 1024] (z1 A/B, z2 A/B); the L3
    `op` accumulators time-share the z1 banks at each bt boundary.
    """
    from concourse import bass, mybir

    f32 = mybir.dt.float32
    f32r = mybir.dt.float32r
    Relu = mybir.ActivationFunctionType.Relu
    Copy = mybir.ActivationFunctionType.Copy
    Alu = mybir.AluOpType

    nc = bass.Bass("TRN2", target_bir_lowering=False, debug=False)

    xa = nc.dram_tensor("xa", [2 * DPC, B], f32r, kind="ExternalInput")
    wpk = nc.dram_tensor("wpk", [P, NG * H], f32r, kind="ExternalInput")
    w2 = nc.dram_tensor("w2", [H, DPC * H], f32r, kind="ExternalInput")
    b2t = nc.dram_tensor("b2t", [H, DPC], f32, kind="ExternalInput")
    w3e = nc.dram_tensor("w3e", [H, DPC * DPC], f32r, kind="ExternalInput")
    o = nc.dram_tensor("o", [NBT, BT // W_, DPC, W_], f32, kind="ExternalOutput")

    # SBUF
    wpk_sb = nc.alloc_sbuf_tensor("wpk_sb", [P, NG * H], f32r)
    w2_sb = nc.alloc_sbuf_tensor("w2_sb", [H, DPC * H], f32r)
    b2_sb = nc.alloc_sbuf_tensor("b2_sb", [H, DPC], f32)
    w3_sb = nc.alloc_sbuf_tensor("w3_sb", [H, DPC * DPC], f32r)
    xo = [nc.alloc_sbuf_tensor(f"xo{i}", [P, BT], f32r) for i in range(2)]
    h1b = [nc.alloc_sbuf_tensor(f"h1b{i}", [P, 2 * W_], f32r) for i in range(4)]
    h2b = [
        [nc.alloc_sbuf_tensor(f"h2b{wp}_{d}", [P, 2 * W_], f32r) for d in range(DPC)]
        for wp in range(2)
    ]
    ost = [nc.alloc_sbuf_tensor(f"ost{i}", [DPC, W_], f32) for i in range(4)]

    # PSUM: zb0/zb1 = z1 ping-pong (+ L3 op at bt ends), zb2/zb3 = z2 ping-pong
    zb = [nc.alloc_psum_tensor(f"zb{i}", [P, 2 * W_], f32) for i in range(4)]

    # semaphores
    s_wdma = nc.alloc_semaphore("s_wdma")
    s_x = [nc.alloc_semaphore(f"s_x{i}") for i in range(2)]
    s_od = [nc.alloc_semaphore(f"s_od{i}") for i in range(4)]
    s_z1 = nc.alloc_semaphore("s_z1")
    s_z2 = nc.alloc_semaphore("s_z2")
    s_op = nc.alloc_semaphore("s_op")
    s_h1 = {"a": nc.alloc_semaphore("s_h1a"), "d": nc.alloc_semaphore("s_h1d")}
    s_h2 = {"a": nc.alloc_semaphore("s_h2a"), "d": nc.alloc_semaphore("s_h2d")}
    s_oc = {"a": nc.alloc_semaphore("s_oca"), "d": nc.alloc_semaphore("s_ocd")}

    NU = NBT * NG * 2  # 64 units; unit u = (bt, g, wp)

    # Chain-to-engine mapping: fill index j (j = 2u + q for z1, 2u + s for
    # z2) has parity-based ownership: even -> ACT ("a"), odd -> DVE ("d").
    # Each engine serves its chains strictly in order, so the engine's
    # counting semaphore value for evict j is simply j//2 + 1.
    def ev_eng(j):
        return "a" if j % 2 == 0 else "d"

    with nc.Block() as block:

        @block.sync
        def _(sp):
            sp.dma_start(wpk_sb[:, :], wpk[:, :]).then_inc(s_wdma, 16)
            sp.dma_start(w2_sb[:, :], w2[:, :]).then_inc(s_wdma, 16)
            sp.dma_start(b2_sb[:, :], b2t[:, :]).then_inc(s_wdma, 16)
            sp.dma_start(w3_sb[:, :], w3e[:, :]).then_inc(s_wdma, 16)
            for bt in range(NBT + 1):
                if bt < NBT:
                    for g in range(NG):
                        idx = bt * NG + g
                        xi = idx % 2
                        if idx >= 2:
                            # xo[xi] last read by L1 fills of (bt,g)-2:
                            # those are z1 fills 4*(idx-2)+1 .. 4*(idx-1)
                            sp.wait_ge(s_z1, 4 * (idx - 1))
                        sl = slice(bt * BT, (bt + 1) * BT)
                        sp.dma_start(
                            xo[xi][0:2, :], xa[4 * g : 4 * g + 2, sl]
                        ).then_inc(s_x[xi], 16)
                        sp.dma_start(
                            xo[xi][32:34, :], xa[4 * g + 2 : 4 * g + 4, sl]
                        ).then_inc(s_x[xi], 16)
                # out DMAs of the previous bt (out-copies all run on ACT)
                if bt >= 1:
                    for w in range(4):
                        k = (bt - 1) * 4 + w
                        sp.wait_ge(s_oc["a"], k + 1)
                        sp.dma_start(o[bt - 1, w], ost[k % 4][:, :]).then_inc(
                            s_od[k % 4], 16
                        )

        UPB = NG * 2  # units per bt

        ENG_OF = ("a", "d")

        def pe_z1_fill(pe, u, q):
            # one z1 fill (unit u, window-pair column q) into zb[q]
            bt, r = divmod(u, UPB)
            g, wp = r // 2, r % 2
            idx = bt * NG + g
            xi = idx % 2
            if wp == 0 and q == 0:
                pe.wait_ge(s_x[xi], 32 * (idx // 2 + 1))
            mm = None
            for s in range(2):
                mm = pe.matmul(
                    zb[q][:, s * W_ : (s + 1) * W_],
                    lhsT=wpk_sb[32 * s : 32 * s + 2, g * H : (g + 1) * H],
                    rhs=xo[xi][32 * s : 32 * s + 2, w_slice(wp, q)],
                    start=True,
                    stop=True,
                    tile_position=(32 * s, 0),
                )
                if u >= 1:
                    # WAR: the s-slice of fill 2(u-1)+q was evicted by
                    # engine s's half-evict
                    mm._wait_ge(s_h1[ENG_OF[s]], 2 * (u - 1) + q + 1)
            mm.then_inc(s_z1, 1)

        def pe_z2_fill(pe, v, s):
            bt, r = divmod(v, UPB)
            g, wp = r // 2, r % 2
            di = 2 * g + s
            mm = None
            for q in range(2):
                mm = pe.matmul(
                    zb[2 + s][:, q * W_ : (q + 1) * W_],
                    lhsT=w2_sb[:, di * H : (di + 1) * H],
                    rhs=h1b[(v % 2) * 2 + q][:, s * W_ : (s + 1) * W_],
                    start=True,
                    stop=True,
                )
                # ready: h1b fill 2v+q's s-half (engine s) evicted
                mm._wait_ge(s_h1[ENG_OF[s]], 2 * v + q + 1)
            mm.then_inc(s_z2, 1)

        def pe_l3_phase(pe, bt):
            # op(w) lives in zb[2 + w % 2][0:8, (w // 2)*512 :] — the z2
            # banks, so the next bt's z1 chains flow undisturbed.
            pe.wait_ge(s_h2["a"], 2 * UPB * (bt + 1))
            pe.wait_ge(s_h2["d"], 2 * UPB * (bt + 1))
            for w in range(4):
                wp, q = w // 2, w % 2
                opv = zb[2 + w % 2][0:DPC, (w // 2) * W_ : (w // 2 + 1) * W_]
                mm = None
                for di in range(DPC):
                    mm = pe.matmul(
                        opv,
                        lhsT=w3_sb[:, DPC * di : DPC * (di + 1)],
                        rhs=h2b[wp][di][:, q * W_ : (q + 1) * W_],
                        start=(di == 0),
                        stop=(di == DPC - 1),
                    )
                mm.then_inc(s_op, 1)

        @block.tensor
        def _(pe):
            pe.wait_ge(s_wdma, 64)
            for t in range(NU + 1):
                # slot t (spread order): zb0 fill early, z2 fills mid,
                # L3 phase at bt boundaries, zb1 fill late.
                if t < NU:
                    pe_z1_fill(pe, t, 0)
                if t >= 1:
                    v = t - 1
                    if v >= 1:
                        # zb2/zb3 WAR: both half-evicts of fills 2(v-1)+s
                        pe.wait_ge(s_h2["a"], 2 * v)
                        pe.wait_ge(s_h2["d"], 2 * v)
                    if v % UPB == 0 and v // UPB > 0:
                        # zb2/zb3 op regions read by out-copies of prev bt
                        pe.wait_ge(s_oc["a"], 4 * (v // UPB))
                    pe_z2_fill(pe, v, 0)
                    pe_z2_fill(pe, v, 1)
                if t < NU:
                    pe_z1_fill(pe, t, 1)
                if t >= 1 and t % UPB == 0:
                    # L3 after the trailing z1 fill so both evictors have
                    # h1 work queued while PE runs the 32 op matmuls
                    pe_l3_phase(pe, t // UPB - 1)

        # Each eviction is split in half along the free dim: ACT does
        # [:, 0:512], DVE does [:, 512:1024], concurrently. Engine sem
        # count for fill j is then j+1 on BOTH s_h1a/s_h1d (resp. h2).
        def ev_h1_half(eng, mine, u, q):
            par = 0 if mine == "a" else 1
            j = 2 * u + q
            hs = slice(par * W_, (par + 1) * W_)
            if u >= 2:
                # h1b[(u%2)*2+q] last read by L2 fills of unit u-2
                eng.wait_ge(s_z2, 2 * (u - 2) + 2)
            dst = h1b[(u % 2) * 2 + q][:, hs]
            ins = (
                eng.activation(dst, zb[q][:, hs], Relu)
                if mine == "a"
                else eng.tensor_scalar(dst, zb[q][:, hs], 0.0, None, Alu.max)
            )
            ins._wait_ge(s_z1, j + 1)
            ins.then_inc(s_h1[mine], 1)

        def ev_h2_half(eng, mine, v, s):
            par = 0 if mine == "a" else 1
            bt, r = divmod(v, NG * 2)
            g, wp = r // 2, r % 2
            j = 2 * v + s
            di = 2 * g + s
            hs = slice(par * W_, (par + 1) * W_)
            if bt > 0 and r == 0 and s == 0:
                eng.wait_ge(s_op, 4 * bt)  # h2b reuse WAR
            dst = h2b[wp][di][:, hs]
            ins = (
                eng.activation(dst, zb[2 + s][:, hs], Relu, bias=b2_sb[:, di : di + 1])
                if mine == "a"
                else eng.tensor_scalar(
                    dst,
                    zb[2 + s][:, hs],
                    b2_sb[:, di : di + 1],
                    0.0,
                    Alu.add,
                    Alu.max,
                )
            )
            ins._wait_ge(s_z2, j + 1)
            ins.then_inc(s_h2[mine], 1)

        def evict_stream(eng, mine):
            eng.wait_ge(s_wdma, 64)
            for t in range(NU + 1):
                if t < NU:
                    ev_h1_half(eng, mine, t, 0)
                if t >= 1:
                    ev_h2_half(eng, mine, t - 1, 0)
                    ev_h2_half(eng, mine, t - 1, 1)
                if t < NU:
                    ev_h1_half(eng, mine, t, 1)
                if t >= 1 and t % (NG * 2) == 0 and mine == "a":
                    bt = t // (NG * 2) - 1
                    for w in range(4):
                        k = bt * 4 + w
                        if k >= 4:
                            eng.wait_ge(s_od[k % 4], 16 * (k // 4))
                        opv = zb[2 + w % 2][
                            0:DPC, (w // 2) * W_ : (w // 2 + 1) * W_
                        ]
                        ins = eng.activation(ost[k % 4][:, :], opv, Copy)
                        ins._wait_ge(s_op, k + 1)
                        ins.then_inc(s_oc["a"], 1)

        @block.scalar
        def _(act):
            evict_stream(act, "a")

        @block.vector
        def _(dve):
            evict_stream(dve, "d")

    return nc


def w_slice(wp, q):
    w = 2 * wp + q
    return slice(w * W_, (w + 1) * W_)


def _prep_inputs(x, W1, b1, W2, b2, W3):
    """Build the per-core input maps (host-side shard + layout transforms)."""
    in_maps = []
    for c in range(NCORES):
        dlo = c * DPC
        dc = slice(dlo, dlo + DPC)

        xa = np.empty((2 * DPC, B), np.float32)
        xa[0::2] = x.T[dc]
        xa[1::2] = 1.0

        wpk = np.zeros((P, NG * H), np.float32)
        for g in range(NG):
            for s in range(2):
                d = dlo + 2 * g + s
                wpk[32 * s, g * H : (g + 1) * H] = W1[d]
                wpk[32 * s + 1, g * H : (g + 1) * H] = b1[d]

        w2c = np.ascontiguousarray(
            W2[dc].transpose(1, 0, 2).reshape(H, DPC * H)
        ).astype(np.float32)

        w3e = np.zeros((H, DPC * DPC), np.float32)
        for i in range(DPC):
            w3e[:, DPC * i + i] = W3[dlo + i]

        in_maps.append(
            {
                "xa": xa,
                "wpk": wpk,
                "w2": w2c,
                "b2t": np.ascontiguousarray(b2[dc].T).astype(np.float32),
                "w3e": w3e,
            }
        )
    return in_maps


def run_on_hw(in_maps, trace=False):
    from concourse.bass_utils import run_bass_kernel_spmd

    if "nc" not in _CACHE:
        _CACHE["nc"] = _build_program_raw()
    nc = _CACHE["nc"]
    res = run_bass_kernel_spmd(
        nc, in_maps, list(range(NCORES)), trace=trace
    )
    return res


def _gather(results, b3):
    out = np.empty((B, D), np.float32)
    for c in range(NCORES):
        dlo = c * DPC
        # o is [bt, w, d, 512] -> [d, B]
        oc = results[c]["o"].transpose(2, 0, 1, 3).reshape(DPC, B)
        out[:, dlo : dlo + DPC] = (oc + b3[dlo : dlo + DPC][:, None]).T
    return out


def kernel(x, W1, b1, W2, b2, W3, b3):
    x = np.asarray(x, np.float32)
    W1 = np.asarray(W1, np.float32)
    b1 = np.asarray(b1, np.float32)
    W2 = np.asarray(W2, np.float32)
    b2 = np.asarray(b2, np.float32)
    W3 = np.asarray(W3, np.float32)
    b3 = np.asarray(b3, np.float32)

    in_maps = _prep_inputs(x, W1, b1, W2, b2, W3)
    res = run_on_hw(in_maps)
    return _gather(res.results, b3)



# revision 2
# speedup vs baseline: 1.0002x; 1.0002x over previous
"""Trainium2 Bass kernel v2 for per-component tiny-MLP stack (CFCN).

Computation (reference):
    h1 = relu(x[:, :, None] * W1 + b1)              # [B, D, H]
    h2 = relu(einsum('bdh,dhk->bdk', h1, W2) + b2)  # [B, D, H]
    out = einsum('bdh,dh->bd', h2, W3) + b3         # [B, D]

B=16384, D=64, H=128. Sharded over 8 NeuronCores by component (8 per core).

v2 dataflow (per core), using the identity
    relu(W1*x + b1) = |W1| * (max(sign(W1)*x, -b') + b'),  b' = b1/|W1|
with W2' = W2*|W1| and b2'' = b2 + W2'^T b' folded on the host:
    z2 = W2'^T max(sign(W1)*x, -b') + b2''

Per (chunk, comp) unit:
  1. one DMA: broadcast x_d row (stride-0 DRAM AP) -> xs [128, len] fp16
  2. DVE 4x: h1'' = (xs * sgn_col) max negb_col -> bf16
  3. PE: z2 = W2'^T h1''  (bf16, N=512 fills, PSUM pairs in 3 rotating
     [128, 1024] tiles)
  4. ACT/DVE split: h2 = relu(z2 + b2'') evict [128, 1024] -> SBUF bf16
  5. PE: out columns via h2-as-stationary matmuls, N=2 (ISA min), batch on
     partitions: opt[:, 2cc:2cc+2] = h2-block^T @ [w3_d, 0]
  6. ACT copies opt (even columns) -> ost; ACT-issued DMA to DRAM.
b3 added on host. First/last chunks use half-size units to shorten the
pipeline lead-in and tail.
"""

import sys

if "/opt/trn_rl_repo" not in sys.path:
    sys.path.insert(0, "/opt/trn_rl_repo")

import numpy as np

B, D, H = 16384, 64, 128
NCORES = 8
DPC = D // NCORES      # components per core = 8
P = 128
C = 4096               # batch chunk (per out-tile)
NC = B // C            # 4 chunks
W2N = 512              # L2 moving window (ISA cap: 512 elements)
SCC = C // P           # 32 L3 sub-chunks per chunk
_CACHE = {}

# unit table: (chunk n, comp d, batch offset within chunk, length)
# first and last chunks use half units to shorten pipeline lead-in/tail
UNITS = []
for n in range(NC):
    for d in range(DPC):
        if n == NC - 1 and d >= DPC - 2:
            UNITS.append((n, d, 0, C // 2))
            UNITS.append((n, d, C // 2, C // 2))
        else:
            UNITS.append((n, d, 0, C))
NU = len(UNITS)

# cumulative counters
_fills = [ln // W2N for (_, _, _, ln) in UNITS]
_pairs = [ln // (2 * W2N) for (_, _, _, ln) in UNITS]
FB = [0]
PB = [0]
for i in range(NU):
    FB.append(FB[-1] + _fills[i])
    PB.append(PB[-1] + _pairs[i])
NPAIR = PB[-1]
CHUNK_START = [next(i for i, u in enumerate(UNITS) if u[0] == n) for n in range(NC)]
CHUNK_END = [max(i for i, u in enumerate(UNITS) if u[0] == n) for n in range(NC)]


def _ev_on_dve(e):
    return e % 3 == 0


def _dve_cnt(e):  # DVE evictions among pairs 0..e
    return e // 3 + 1


def _act_cnt(e):  # ACT evictions among pairs 0..e
    return (e + 1) - (e // 3 + 1)


def _build_v2():
    from concourse import bass, mybir

    f32 = mybir.dt.float32
    bf16 = mybir.dt.bfloat16
    fp16 = mybir.dt.float16
    Relu = mybir.ActivationFunctionType.Relu
    Copy = mybir.ActivationFunctionType.Copy
    Alu = mybir.AluOpType

    nc = bass.Bass("TRN2", target_bir_lowering=False, debug=False)

    xb = nc.dram_tensor("xb", [DPC, B], fp16, kind="ExternalInput")
    w2p = nc.dram_tensor("w2p", [H, DPC * H], bf16, kind="ExternalInput")
    sgn = nc.dram_tensor("sgn", [H, DPC], f32, kind="ExternalInput")
    nbp = nc.dram_tensor("nbp", [H, DPC], f32, kind="ExternalInput")
    b2t = nc.dram_tensor("b2t", [H, DPC], f32, kind="ExternalInput")
    w3t = nc.dram_tensor("w3t", [H, 2 * DPC], bf16, kind="ExternalInput")
    o = nc.dram_tensor("o", [NC, P, SCC * DPC], f32, kind="ExternalOutput")

    w2p_sb = nc.alloc_sbuf_tensor("w2p_sb", [H, DPC * H], bf16)
    sgn_sb = nc.alloc_sbuf_tensor("sgn_sb", [H, DPC], f32)
    nbp_sb = nc.alloc_sbuf_tensor("nbp_sb", [H, DPC], f32)
    b2_sb = nc.alloc_sbuf_tensor("b2_sb", [H, DPC], f32)
    w3_sb = nc.alloc_sbuf_tensor("w3_sb", [H, 2 * DPC], bf16)
    xs = [nc.alloc_sbuf_tensor(f"xs{i}", [P, C], fp16) for i in range(4)]
    h1p = [nc.alloc_sbuf_tensor(f"h1p{i}", [P, C], bf16) for i in range(4)]
    h2 = [nc.alloc_sbuf_tensor(f"h2_{i}", [P, C], bf16) for i in range(2)]
    ost = [nc.alloc_sbuf_tensor(f"ost{i}", [P, SCC * DPC], f32) for i in range(2)]

    z2t = [nc.alloc_psum_tensor(f"z2t{j}", [P, 2 * W2N], f32) for j in range(3)]
    opt = nc.alloc_psum_tensor("opt", [P, 2 * SCC * DPC], f32)

    s_w = nc.alloc_semaphore("s_w")
    s_x = nc.alloc_semaphore("s_x")
    s_gen = nc.alloc_semaphore("s_gen")
    s_l2 = nc.alloc_semaphore("s_l2")
    s_eva = nc.alloc_semaphore("s_eva")
    s_evd = nc.alloc_semaphore("s_evd")
    s_l3 = nc.alloc_semaphore("s_l3")
    s_oc = nc.alloc_semaphore("s_oc")
    s_od = nc.alloc_semaphore("s_od")

    with nc.Block() as block:

        @block.sync
        def _(sp):
            sp.dma_start(sgn_sb[:, :], sgn[:, :]).then_inc(s_w, 16)
            sp.dma_start(nbp_sb[:, :], nbp[:, :]).then_inc(s_w, 16)
            for i, (n, d, off, ln) in enumerate(UNITS):
                if i == 1:
                    sp.dma_start(w2p_sb[:, :], w2p[:, :]).then_inc(s_w, 16)
                    sp.dma_start(b2_sb[:, :], b2t[:, :]).then_inc(s_w, 16)
                    sp.dma_start(w3_sb[:, :], w3t[:, :]).then_inc(s_w, 16)
                if i >= 4:
                    sp.wait_ge(s_gen, i - 3)  # xs[i%4] read by gen(i-4)
                lo = n * C + off
                sp.dma_start(
                    xs[i % 4][:, 0:ln],
                    xb[d : d + 1, lo : lo + ln].partition_broadcast(P).squeeze(1),
                ).then_inc(s_x, 16)

        @block.vector
        def _(dve):
            dve.wait_ge(s_w, 32)
            for i in range(NU + 1):
                if i < NU:
                    n, d, off, ln = UNITS[i]
                    dve.wait_ge(s_x, 16 * (i + 1))
                    if i >= 4:
                        dve.wait_ge(s_l2, FB[i - 3])  # h1p[i%4] read by L2(i-4)
                    dve.tensor_scalar(
                        h1p[i % 4][:, 0:ln],
                        xs[i % 4][:, 0:ln],
                        sgn_sb[:, d : d + 1],
                        nbp_sb[:, d : d + 1],
                        Alu.mult,
                        Alu.max,
                    ).then_inc(s_gen, 1)
                if i >= 1:
                    v = i - 1
                    vn, vd, voff, vln = UNITS[v]
                    for w in range(vln // (2 * W2N)):
                        e = PB[v] + w
                        if not _ev_on_dve(e):
                            continue
                        dve.wait_ge(s_l2, 2 * (e + 1))
                        if w == 0 and v >= 2:
                            dve.wait_ge(s_l3, v - 1)  # h2[v%2] read by L3(v-2)
                        dve.tensor_scalar(
                            h2[v % 2][:, w * 1024 : (w + 1) * 1024],
                            z2t[e % 3][:, :],
                            b2_sb[:, vd : vd + 1],
                            0.0,
                            Alu.add,
                            Alu.max,
                        ).then_inc(s_evd, 1)

        @block.scalar
        def _(act):
            act.wait_ge(s_w, 80)
            for i in range(NU + 1):
                if i >= 1:
                    v = i - 1
                    vn, vd, voff, vln = UNITS[v]
                    for w in range(vln // (2 * W2N)):
                        e = PB[v] + w
                        if _ev_on_dve(e):
                            continue
                        act.wait_ge(s_l2, 2 * (e + 1))
                        if w == 0 and v >= 2:
                            act.wait_ge(s_l3, v - 1)
                        act.activation(
                            h2[v % 2][:, w * 1024 : (w + 1) * 1024],
                            z2t[e % 3][:, :],
                            Relu,
                            bias=b2_sb[:, vd : vd + 1],
                        ).then_inc(s_eva, 1)
                    # ost-copy + out-DMA for the chunk ending at unit v-1
                    # (issued one unit later to avoid the PE->ACT cycle)
                    if v >= 1 and (v - 1) in CHUNK_END:
                        nn = UNITS[v - 1][0]
                        act.wait_ge(s_l3, v)  # all units of chunk nn done
                        if nn >= 2:
                            act.wait_ge(s_od, 16 * (nn - 1))  # ost[nn%2] reuse
                        act.activation(
                            ost[nn % 2][:, :],
                            opt[:, :].rearrange("p (c two) -> p c two", two=2)[
                                :, :, 0
                            ],
                            Copy,
                        ).then_inc(s_oc, 1)
                        act.dma_start(o[nn], ost[nn % 2][:, :]).then_inc(s_od, 16)
            # tail: last chunk
            act.wait_ge(s_l3, NU)
            if NC >= 3:
                act.wait_ge(s_od, 16 * (NC - 2))
            act.activation(
                ost[(NC - 1) % 2][:, :],
                opt[:, :].rearrange("p (c two) -> p c two", two=2)[:, :, 0],
                Copy,
            ).then_inc(s_oc, 1)
            act.dma_start(o[NC - 1], ost[(NC - 1) % 2][:, :]).then_inc(s_od, 16)

        @block.tensor
        def _(pe):
            pe.wait_ge(s_w, 80)
            for i in range(NU + 1):
                if i < NU:
                    n, d, off, ln = UNITS[i]
                    pe.wait_ge(s_gen, i + 1)
                    for w in range(ln // W2N):
                        f = FB[i] + w
                        p_, half = divmod(f, 2)
                        if p_ >= 3 and half == 0:
                            ep = p_ - 3
                            if _ev_on_dve(ep):
                                pe.wait_ge(s_evd, _dve_cnt(ep))
                            else:
                                pe.wait_ge(s_eva, _act_cnt(ep))
                        pe.matmul(
                            z2t[p_ % 3][:, half * W2N : (half + 1) * W2N],
                            lhsT=w2p_sb[:, d * H : (d + 1) * H],
                            rhs=h1p[i % 4][:, w * W2N : (w + 1) * W2N],
                            start=True,
                            stop=True,
                        ).then_inc(s_l2, 1)
                if i >= 1:
                    v = i - 1
                    vn, vd, voff, vln = UNITS[v]
                    e_first, e_last = PB[v], PB[v + 1] - 1
                    dv = [e for e in range(e_first, e_last + 1) if _ev_on_dve(e)]
                    av = [e for e in range(e_first, e_last + 1) if not _ev_on_dve(e)]
                    if dv:
                        pe.wait_ge(s_evd, _dve_cnt(dv[-1]))
                    if av:
                        pe.wait_ge(s_eva, _act_cnt(av[-1]))
                    if v in CHUNK_START and vn >= 1:
                        pe.wait_ge(s_oc, vn)  # opt read by ost-copy(vn-1)
                    mm = None
                    for c in range(vln // P):
                        cc = (voff // P + c) * DPC + vd
                        mm = pe.matmul(
                            opt[:, 2 * cc : 2 * cc + 2],
                            lhsT=h2[v % 2][:, c * P : (c + 1) * P],
                            rhs=w3_sb[:, 2 * vd : 2 * vd + 2],
                            start=True,
                            stop=True,
                        )
                    mm.then_inc(s_l3, 1)

    return nc


def _prep_inputs(x, W1, b1, W2, b2, W3):
    """Host-side shard + fold transforms."""
    import ml_dtypes

    bf = ml_dtypes.bfloat16
    in_maps = []
    for cidx in range(NCORES):
        dlo = cidx * DPC
        xbm = np.empty((DPC, B), np.float16)
        w2pm = np.empty((H, DPC * H), np.float64)
        sgnm = np.empty((H, DPC), np.float32)
        nbpm = np.empty((H, DPC), np.float32)
        b2c = np.empty((H, DPC), np.float64)
        w3c = np.zeros((H, 2 * DPC), np.float32)
        for d in range(DPC):
            dg = dlo + d
            w1 = W1[dg].astype(np.float64)
            b1d = b1[dg].astype(np.float64)
            aw = np.abs(w1)
            tiny = aw < 1e-12
            aws = np.where(tiny, 1.0, aw)
            bprime = np.where(tiny, 0.0, b1d / aws)
            sg = np.where(w1 >= 0, 1.0, -1.0)
            w2d = W2[dg].astype(np.float64)
            w2dp = w2d * np.where(tiny, 0.0, aw)[:, None]
            fold = (w2dp * bprime[:, None]).sum(axis=0)
            const = (w2d[tiny] * np.maximum(b1d[tiny], 0.0)[:, None]).sum(axis=0)
            xbm[d] = x[:, dg].astype(np.float16)
            sgnm[:, d] = sg
            nbpm[:, d] = -bprime
            w2pm[:, d * H : (d + 1) * H] = w2dp
            b2c[:, d] = b2[dg].astype(np.float64) + fold + const
            w3c[:, 2 * d] = W3[dg]
        in_maps.append(
            {
                "xb": xbm,
                "w2p": w2pm.astype(np.float32).astype(bf),
                "sgn": sgnm,
                "nbp": nbpm,
                "b2t": b2c.astype(np.float32),
                "w3t": w3c.astype(bf),
            }
        )
    return in_maps


def run_on_hw(in_maps, trace=False):
    from concourse.bass_utils import run_bass_kernel_spmd

    if "nc" not in _CACHE:
        _CACHE["nc"] = _build_v2()
    nc = _CACHE["nc"]
    res = run_bass_kernel_spmd(nc, in_maps, list(range(NCORES)), trace=trace)
    return res


def _gather(results, b3):
    out = np.empty((B, D), np.float32)
    for cidx in range(NCORES):
        dlo = cidx * DPC
        oc = np.asarray(results[cidx]["o"], np.float32)  # [NC, P, SCC*DPC]
        oc = oc.reshape(NC, P, SCC, DPC)                 # col cc = c*DPC + d
        ob = oc.transpose(0, 2, 1, 3).reshape(B, DPC)    # b = n*C + c*P + p
        out[:, dlo : dlo + DPC] = ob + b3[dlo : dlo + DPC][None, :]
    return out


def kernel(x, W1, b1, W2, b2, W3, b3):
    x = np.asarray(x, np.float32)
    in_maps = _prep_inputs(
        x,
        np.asarray(W1, np.float32),
        np.asarray(b1, np.float32),
        np.asarray(W2, np.float32),
        np.asarray(b2, np.float32),
        np.asarray(W3, np.float32),
    )
    res = run_on_hw(in_maps)
    return _gather(res.results, np.asarray(b3, np.float32))
